# revision 1
# baseline (speedup 1.0000x reference)
"""Self-contained DKVMN Bass kernel (graded entry: kernel(**inputs)) for TRN2, 8-core data-parallel over batch.

Layouts (per core, B_loc=16, S=200, DK=128, DV=64):
  l = b*S + s  (flattened lookup index, 0..3199), p = l % 128, j = l // 128
  k_sb/v_sb   [p, (j, k)]     gathered embedding rows
  k_t/v_t     [k, l]          transposed (PE-friendly)
  w_sb        [p, (j, v)]     softmax attention weights
  e_t/a_t     [k, l]          erase/add vectors (transposed layout)
  Scan layout: partition q = b*8 + k_hi (k_hi in 0..7), free = k_lo*64 + v
  (k = k_hi*16 + k_lo), i.e. M_sb[q, k_lo*64+v] = Mv[b, v, k_hi*16+k_lo]
  e_c/a_c     [q, (t, k_lo)]
  w_rep       [q, (t, v)]     (replicated over k_hi groups)
  reads_sb    [q, (t, k_lo)]
"""

import numpy as np

import concourse.bacc as bacc
import concourse.bass as bass
import concourse.mybir as mybir
from concourse.tile import TileContext
from concourse.masks import make_identity

F32 = mybir.dt.float32
I32 = mybir.dt.int32
AX = mybir.AxisListType
ALU = mybir.AluOpType
ACTF = mybir.ActivationFunctionType

B, S, DK, DV, NQ = 128, 200, 128, 64, 10000
NC = 8
BL = B // NC          # 16 batches per core
L = BL * S            # 3200 lookups per core
NJ = L // 128         # 25 gather groups
CH = 400              # matmul free-dim chunk (<=512)
NCH = L // CH         # 8 chunks


def build_kernel():
    nc = bacc.Bacc("TRN2", target_bir_lowering=False, debug=False, num_devices=NC)

    # ---- I/O ----
    qidx = nc.dram_tensor("qidx", [128, NJ], I32, kind="ExternalInput").ap()
    xidx = nc.dram_tensor("xidx", [128, NJ], I32, kind="ExternalInput").ap()
    k_emb = nc.dram_tensor("k_emb", [NQ, DK], F32, kind="ExternalInput").ap()
    v_emb = nc.dram_tensor("v_emb", [2 * NQ, DK], F32, kind="ExternalInput").ap()
    MkT = nc.dram_tensor("MkT", [DK, DV], F32, kind="ExternalInput").ap()
    eW = nc.dram_tensor("eW", [DK, DK], F32, kind="ExternalInput").ap()
    aW = nc.dram_tensor("aW", [DK, DK], F32, kind="ExternalInput").ap()
    fWr = nc.dram_tensor("fWr", [DK, DK], F32, kind="ExternalInput").ap()
    fWk = nc.dram_tensor("fWk", [DK, DK], F32, kind="ExternalInput").ap()
    pW = nc.dram_tensor("pW", [DK, 1], F32, kind="ExternalInput").ap()
    eb = nc.dram_tensor("eb", [DK, 1], F32, kind="ExternalInput").ap()
    ab = nc.dram_tensor("ab", [DK, 1], F32, kind="ExternalInput").ap()
    fb = nc.dram_tensor("fb", [DK, 1], F32, kind="ExternalInput").ap()
    pb = nc.dram_tensor("pb", [1, 1], F32, kind="ExternalInput").ap()
    M0c = nc.dram_tensor("M0c", [128, 1024], F32, kind="ExternalInput").ap()
    out = nc.dram_tensor("out", [1, L], F32, kind="ExternalOutput").ap()

    # ---- DRAM scratch for relayouts ----
    e_dram = nc.dram_tensor("e_scr", [DK, L], F32).ap()
    a_dram = nc.dram_tensor("a_scr", [DK, L], F32).ap()
    w_dram = nc.dram_tensor("w_scr", [L, DV], F32).ap()
    r_dram = nc.dram_tensor("r_scr", [BL * S, DK], F32).ap()

    with TileContext(nc) as tc:
        with (
            tc.tile_pool(name="persist", bufs=1) as pp,
            tc.tile_pool(name="work", bufs=2) as wp,
            tc.tile_pool(name="state", bufs=2) as sp,
            tc.tile_pool(name="scant", bufs=1) as scp,
            tc.tile_pool(name="psum", bufs=2, space="PSUM") as pu,
            tc.tile_pool(name="psum_t", bufs=2, space="PSUM") as put,
        ):
            # ---------- params to SBUF ----------
            ident = pp.tile([128, 128], F32)
            make_identity(nc, ident[:])
            MkT_sb = pp.tile([DK, DV], F32)
            nc.sync.dma_start(out=MkT_sb[:], in_=MkT)
            eW_sb = pp.tile([DK, DK], F32)
            nc.sync.dma_start(out=eW_sb[:], in_=eW)
            aW_sb = pp.tile([DK, DK], F32)
            nc.sync.dma_start(out=aW_sb[:], in_=aW)
            fWr_sb = pp.tile([DK, DK], F32)
            nc.sync.dma_start(out=fWr_sb[:], in_=fWr)
            fWk_sb = pp.tile([DK, DK], F32)
            nc.sync.dma_start(out=fWk_sb[:], in_=fWk)
            pW_sb = pp.tile([DK, 1], F32)
            nc.sync.dma_start(out=pW_sb[:], in_=pW)
            eb_sb = pp.tile([DK, 1], F32)
            nc.sync.dma_start(out=eb_sb[:], in_=eb)
            ab_sb = pp.tile([DK, 1], F32)
            nc.sync.dma_start(out=ab_sb[:], in_=ab)
            fb_sb = pp.tile([DK, 1], F32)
            nc.sync.dma_start(out=fb_sb[:], in_=fb)
            pb_sb = pp.tile([1, 1], F32)
            nc.sync.dma_start(out=pb_sb[:], in_=pb)

            qidx_sb = pp.tile([128, NJ], I32)
            nc.sync.dma_start(out=qidx_sb[:], in_=qidx)
            xidx_sb = pp.tile([128, NJ], I32)
            nc.sync.dma_start(out=xidx_sb[:], in_=xidx)

            # ---------- gather + transpose ----------
            k_t = pp.tile([128, L], F32)   # [k, l]
            vt_pool = tc.tile_pool(name="vt", bufs=1)
            vtp = vt_pool.__enter__()
            v_t = vtp.tile([128, L], F32)  # [k, l]
            for j in range(NJ):
                ksl = wp.tile([128, 128], F32, tag="gk")
                nc.gpsimd.indirect_dma_start(
                    out=ksl[:],
                    out_offset=None,
                    in_=k_emb,
                    in_offset=bass.IndirectOffsetOnAxis(ap=qidx_sb[:, j : j + 1], axis=0),
                )
                tp = put.tile([128, 128], F32, tag="tr")
                nc.tensor.transpose(out=tp[:], in_=ksl[:], identity=ident[:])
                nc.scalar.copy(out=k_t[:, j * 128 : (j + 1) * 128], in_=tp[:])

                vsl = wp.tile([128, 128], F32, tag="gv")
                nc.gpsimd.indirect_dma_start(
                    out=vsl[:],
                    out_offset=None,
                    in_=v_emb,
                    in_offset=bass.IndirectOffsetOnAxis(ap=xidx_sb[:, j : j + 1], axis=0),
                )
                tp2 = put.tile([128, 128], F32, tag="tr")
                nc.tensor.transpose(out=tp2[:], in_=vsl[:], identity=ident[:])
                nc.scalar.copy(out=v_t[:, j * 128 : (j + 1) * 128], in_=tp2[:])

            # ---------- w = softmax(k @ Mk^T) ----------
            w_sb = pp.tile([128, NJ * DV], F32)  # [p, (j, v)]
            for j in range(NJ):
                wps = pu.tile([128, DV], F32, tag="mm")
                nc.tensor.matmul(
                    out=wps[:],
                    lhsT=k_t[:, j * 128 : (j + 1) * 128],
                    rhs=MkT_sb[:],
                    start=True,
                    stop=True,
                )
                negmax = wp.tile([128, 1], F32, tag="negmax")
                nc.vector.tensor_reduce(
                    out=negmax[:], in_=wps[:], axis=AX.X, op=ALU.max, negate=True
                )
                expt = wp.tile([128, DV], F32, tag="expt")
                sums = wp.tile([128, 1], F32, tag="sums")
                nc.scalar.activation(
                    out=expt[:], in_=wps[:], func=ACTF.Exp,
                    bias=negmax[:], accum_out=sums[:],
                )
                rsum = wp.tile([128, 1], F32, tag="rsum")
                nc.vector.reciprocal(out=rsum[:], in_=sums[:])
                nc.vector.tensor_scalar_mul(
                    w_sb[:, j * DV : (j + 1) * DV], expt[:], rsum[:, :1]
                )

            # ---------- e = sigmoid(v@eW+eb), a = tanh(v@aW+ab)  -> e_dram/a_dram [k, l] ----------
            for c in range(NCH):
                cs = slice(c * CH, (c + 1) * CH)
                eps = pu.tile([128, CH], F32, tag="mm")
                nc.tensor.matmul(out=eps[:], lhsT=eW_sb[:], rhs=v_t[:, cs], start=True, stop=True)
                ech = wp.tile([128, CH], F32, tag="ech")
                nc.scalar.activation(out=ech[:], in_=eps[:], func=ACTF.Sigmoid, bias=eb_sb[:, :1])
                nc.sync.dma_start(out=e_dram[:, cs], in_=ech[:])
                aps = pu.tile([128, CH], F32, tag="mm")
                nc.tensor.matmul(out=aps[:], lhsT=aW_sb[:], rhs=v_t[:, cs], start=True, stop=True)
                ach = wp.tile([128, CH], F32, tag="ach")
                nc.scalar.activation(out=ach[:], in_=aps[:], func=ACTF.Tanh, bias=ab_sb[:, :1])
                nc.sync.dma_start(out=a_dram[:, cs], in_=ach[:])

            vt_pool.__exit__(None, None, None)

            # ---------- relayout via DRAM ----------
            # w_dram[l, v] from w_sb[p, (j, v)]: l = j*128+p
            nc.sync.dma_start(
                out=w_dram.rearrange("(j p) v -> j p v", p=128).transpose([1, 0, 2]),
                in_=w_sb[:].rearrange("p (j v) -> p j v", v=DV),
            )

            e_c = pp.tile([128, S * 16], F32)   # [q, (t, k_lo)]
            a_c = pp.tile([128, S * 16], F32)
            # per-kh: [b, t, kl] view of e_dram[k= kh*16+kl, l= b*S+t]
            ein = e_dram.rearrange("(kh kl) (b t) -> kh kl b t", kh=8, b=BL).transpose([0, 2, 3, 1])
            ain = a_dram.rearrange("(kh kl) (b t) -> kh kl b t", kh=8, b=BL).transpose([0, 2, 3, 1])
            ecv = e_c[:].rearrange("(b kh) (t kl) -> kh b t kl", kh=8, kl=16)
            acv = a_c[:].rearrange("(b kh) (t kl) -> kh b t kl", kh=8, kl=16)
            for kh in range(8):
                nc.sync.dma_start(out=ecv[kh], in_=ein[kh])
                nc.sync.dma_start(out=acv[kh], in_=ain[kh])

            # w_rep[q=(b,k_hi), (t, v)] = w_dram[b*S+t, v]  (replicated over k_hi)
            w_rep = pp.tile([128, S * DV], F32)
            wview = w_rep[:].rearrange("(b kh) f -> b kh f", kh=8).transpose([1, 0, 2])
            win = w_dram.rearrange("(b t) v -> b (t v)", b=BL)
            for kh in range(8):
                nc.sync.dma_start(out=wview[kh], in_=win)

            # ---------- the scan ----------
            M_cur = sp.tile([128, 1024], F32, tag="M")
            nc.sync.dma_start(out=M_cur[:], in_=M0c)
            reads_sb = pp.tile([128, S * 16], F32)  # [q, (t, k_lo)]

            e3 = e_c[:].rearrange("q (t kl) -> q t kl", kl=16)
            a3 = a_c[:].rearrange("q (t kl) -> q t kl", kl=16)
            w3 = w_rep[:].rearrange("q (t v) -> q t v", v=DV)
            r3 = reads_sb[:].rearrange("q (t kl) -> q t kl", kl=16)

            for t in range(S):
                # broadcast views over free dims: [q, k_lo, v]
                Ev = e3[:, t, :].to_broadcast([128, 16, DV])
                Av = a3[:, t, :].to_broadcast([128, 16, DV])
                Wv = w3[:, t, :].rearrange("q (u v) -> q u v", u=1).to_broadcast([128, 16, DV])
                Mv = M_cur[:].rearrange("q (kl v) -> q kl v", v=DV)

                # read_t = sum_v w * M   (before update)
                rt = scp.tile([128, 1024], F32, tag="rt")
                rt3 = rt[:].rearrange("q (kl v) -> q kl v", v=DV)
                nc.vector.tensor_tensor(out=rt3, in0=Wv, in1=Mv, op=ALU.mult)
                nc.vector.tensor_reduce(out=r3[:, t, :], in_=rt3, axis=AX.X, op=ALU.add)

                # u = (M*E - A) * W ; M' = M - u
                me = scp.tile([128, 1024], F32, tag="me")
                me3 = me[:].rearrange("q (kl v) -> q kl v", v=DV)
                nc.vector.tensor_tensor(out=me3, in0=Mv, in1=Ev, op=ALU.mult)
                mea = scp.tile([128, 1024], F32, tag="mea")
                mea3 = mea[:].rearrange("q (kl v) -> q kl v", v=DV)
                nc.vector.tensor_tensor(out=mea3, in0=me3, in1=Av, op=ALU.subtract)
                u = scp.tile([128, 1024], F32, tag="u")
                u3 = u[:].rearrange("q (kl v) -> q kl v", v=DV)
                nc.vector.tensor_tensor(out=u3, in0=mea3, in1=Wv, op=ALU.mult)
                M_new = sp.tile([128, 1024], F32, tag="M")
                nc.vector.tensor_tensor(out=M_new[:], in0=M_cur[:], in1=u[:], op=ALU.subtract)
                M_cur = M_new

            # ---------- reads relayout: [q,(t,k_lo)] -> r_dram[l, k] -> reads_t [k, l] ----------
            # r_dram[(b t), k] with k = k_hi*16+k_lo ; source partition q=(b,k_hi)
            rdv = r_dram.rearrange("(b t) (kh kl) -> b t kh kl", b=BL, kh=8).transpose([2, 0, 1, 3])
            rsv = reads_sb[:].rearrange("(b kh) (t kl) -> kh b t kl", kh=8, kl=16)
            for kh in range(8):
                nc.sync.dma_start(out=rdv[kh], in_=rsv[kh])
            reads_t = pp.tile([128, L], F32)
            nc.sync.dma_start(
                out=reads_t[:], in_=r_dram.transpose([1, 0])
            )

            # ---------- f = tanh([reads, k] @ fW + fb); p = sigmoid(f@pW+pb) ----------
            pred = pp.tile([1, L], F32)
            for c in range(NCH):
                cs = slice(c * CH, (c + 1) * CH)
                fps = pu.tile([128, CH], F32, tag="mm")
                nc.tensor.matmul(out=fps[:], lhsT=fWr_sb[:], rhs=reads_t[:, cs], start=True, stop=False)
                nc.tensor.matmul(out=fps[:], lhsT=fWk_sb[:], rhs=k_t[:, cs], start=False, stop=True)
                f_sb = wp.tile([128, CH], F32, tag="fsb")
                nc.scalar.activation(out=f_sb[:], in_=fps[:], func=ACTF.Tanh, bias=fb_sb[:, :1])
                pps = pu.tile([1, CH], F32, tag="mm")
                nc.tensor.matmul(out=pps[:], lhsT=pW_sb[:], rhs=f_sb[:], start=True, stop=True)
                nc.scalar.activation(out=pred[:, cs], in_=pps[:], func=ACTF.Sigmoid, bias=pb_sb[:, :1])

            nc.sync.dma_start(out=out, in_=pred[:])

    nc.compile()
    return nc


# ------------------------------------------------------------------
_CACHED = None


def _get_nc():
    global _CACHED
    if _CACHED is None:
        _CACHED = build_kernel()
    return _CACHED


def make_in_maps(question_seq, correct_seq, k_emb, v_emb, Mk, Mv0, fW, fb_, eW, eb_, aW, ab_, pW, pb_):
    q = np.asarray(question_seq).astype(np.int64)
    c = np.asarray(correct_seq).astype(np.int64)
    x = q + NQ * c

    shared = {
        "k_emb": np.ascontiguousarray(np.asarray(k_emb, np.float32)),
        "v_emb": np.ascontiguousarray(np.asarray(v_emb, np.float32)),
        "MkT": np.ascontiguousarray(np.asarray(Mk, np.float32).T),
        "eW": np.ascontiguousarray(np.asarray(eW, np.float32)),
        "aW": np.ascontiguousarray(np.asarray(aW, np.float32)),
        "fWr": np.ascontiguousarray(np.asarray(fW, np.float32)[:DK]),
        "fWk": np.ascontiguousarray(np.asarray(fW, np.float32)[DK:]),
        "pW": np.ascontiguousarray(np.asarray(pW, np.float32).reshape(DK, 1)),
        "eb": np.ascontiguousarray(np.asarray(eb_, np.float32).reshape(DK, 1)),
        "ab": np.ascontiguousarray(np.asarray(ab_, np.float32).reshape(DK, 1)),
        "fb": np.ascontiguousarray(np.asarray(fb_, np.float32).reshape(DK, 1)),
        "pb": np.ascontiguousarray(np.asarray(pb_, np.float32).reshape(1, 1)),
    }
    # M0c[q=(b,kh), kl*64+v] = Mv0[v, kh*16+kl]  (same for every b)
    Mv0 = np.asarray(Mv0, np.float32)  # [DV, DK]
    m0 = Mv0.reshape(DV, 8, 16)                    # [v, kh, kl]
    m0 = np.transpose(m0, (1, 2, 0))               # [kh, kl, v]
    M0c = np.broadcast_to(m0.reshape(1, 8, 16 * DV), (BL, 8, 16 * DV))
    M0c = np.ascontiguousarray(M0c.reshape(128, 1024), np.float32)
    shared["M0c"] = M0c

    in_maps = []
    for core in range(NC):
        bs = slice(core * BL, (core + 1) * BL)
        qf = q[bs].reshape(-1)   # l = b*S + s
        xf = x[bs].reshape(-1)
        qi = np.ascontiguousarray(qf.reshape(NJ, 128).T.astype(np.int32))  # [p, j]
        xi = np.ascontiguousarray(xf.reshape(NJ, 128).T.astype(np.int32))
        m = dict(shared)
        m["qidx"] = qi
        m["xidx"] = xi
        in_maps.append(m)
    return in_maps


def kernel(**inputs):
    from concourse.bass_utils import run_bass_kernel_spmd

    nc = _get_nc()
    in_maps = make_in_maps(
        inputs["question_seq"], inputs["correct_seq"], inputs["k_emb"],
        inputs["v_emb"], inputs["Mk"], inputs["Mv0"], inputs["fW"], inputs["fb"],
        inputs["eW"], inputs["eb"], inputs["aW"], inputs["ab"], inputs["pW"], inputs["pb"],
    )
    res = run_bass_kernel_spmd(nc, in_maps, core_ids=list(range(NC)))
    outs = [r["out"].reshape(BL, S) for r in res.results]
    return np.concatenate(outs, axis=0).astype(np.float32)

